# revision 39
# baseline (speedup 1.0000x reference)
"""Trainium2 Bass kernel for nn_BaseQVLayer (GNN message passing).

Reference computation (single device):
    xp = x @ Wx + bx                      # [Nx, E]
    yp = y @ Wy + by                      # [Ny, E]
    A_ = xp @ yp.T                        # [Nx, Ny]
    A  = 2*A_ / (||xp_i||^2 + ||yp_j||^2) # Dice-style normalization
    gwf = A.T @ xp                        # [Ny, E]
    out = relu(gwf @ Wg + bg)             # [Ny, E]

Distribution: column-parallel over Ny (8 shards of 1024 y-rows, one per
NeuronCore).  Each core needs the *full* xp in two layouts (normal for the
gwf contraction, transposed for the A matmul) plus its own ypT shard.  The
xp/xpT/|xp|^2 pieces are computed per-core for its own Nx shard only and
exchanged with a single packed AllGather; everything downstream is local to
the core, so there is no AllReduce at all.

Precision strategy: every matmul except the small output MLP runs as
fp8-e4m3 DoubleRow matmuls on a hi/lo split of each operand, keeping the
three significant cross terms (hh + hl + lh).  A DoubleRow matmul retires
two 128-deep contraction tiles per instruction at 0.5 PE cycles/row, so the
3-term decomposition costs 0.75x the bf16 instruction stream while carrying
~8 mantissa bits per operand - measured end-to-end relative error ~3e-3,
slightly better than the all-bf16 version.  Operands are pre-scaled into
e4m3's normal range (x*8, W*32, xp*8, a*64) so the lo residuals avoid the
subnormal region; all descales fold into existing activation/DVE scale
parameters for free.  The normalization chain stays fp32.

Per-core phases:
  1. project own shards (12 DoubleRow matmuls per psum group): xp, xpT,
     Dcol, ypT, Drow.  The Drow ones-matmuls are software-pipelined one
     block behind the sqd chain so PE never waits on ACT/DVE.
  2. packed AllGather of (xp_hi, xp_lo, xpT_hi, xpT_lo, Dcol)
  3. shard-rotated passes over a single flat (sub, t) loop: each core
     starts its A/gwf accumulation on its own SBUF-resident shard (hiding
     the AllGather), then walks the other 7 shards via partition_id-offset
     reads of the gathered buffer (prefetched one block ahead).  The
     post-A elementwise chain (d -> 1/d -> a=A_*r -> fp8 hi -> fp8 lo,
     split across ACT/DVE/Pool) takes ~2.4us, so gwf DoubleRow matmuls for
     a t-pair are deferred 3 t-slots; the deferral queue flows across the
     sub boundary and the fused ReLU-MLP of each sub is emitted a few
     t-slots into the next sub, so PE never drains between passes.

kernel(**inputs) takes full unsharded inputs and returns the full output.
"""

import sys

if "/opt/trn_rl_repo" not in sys.path:
    sys.path.insert(0, "/opt/trn_rl_repo")

import numpy as np

NCORES = 8
NX, NY = 8192, 8192
FX, FY = 1024, 1024
EMB, EMB_OUT = 512, 512

P = 128
KT = FX // P           # 8   k-tiles over feature dim
ME = EMB // P          # 4   emb tiles
NSH = NX // NCORES     # 1024 rows per shard
TSH = NSH // P         # 8   nx tiles per shard
TALL = NX // P         # 64  nx tiles total
NYSUB = 512            # ny columns per pass
NSUBS = NSH // NYSUB   # 2   passes

# fp8 pre-scales (keep hi AND lo operands in e4m3's normal range)
SX = 8.0               # raw x / y
SW = 32.0              # Wx / Wy (uniform +-1/32)
SP = 8.0               # xp / yp projections
SA = 64.0              # normalized affinity a

XP_ELEMS = P * TSH * EMB          # 524288 (per hi or lo plane)
XPT_ELEMS = P * ME * NSH          # 524288
DCOL_SLOTS = 4 * P * TSH          # 4096 fp8 slots = 1024 f32 values
XPT_OFF = 2 * XP_ELEMS
DC_OFF = 2 * XP_ELEMS + 2 * XPT_ELEMS
SH_ELEMS = DC_OFF + DCOL_SLOTS    # 2101248 fp8 elements per shard

_CACHE = {}


def _build_nc(with_collective=True, passes_repeat=1):
    import concourse.bass as bass
    from concourse import bacc
    import concourse.mybir as mybir
    import concourse.tile as tile

    F32 = mybir.dt.float32
    BF16 = mybir.dt.bfloat16
    E4 = mybir.dt.float8e4
    ALU = mybir.AluOpType
    ACTF = mybir.ActivationFunctionType
    DR = mybir.MatmulPerfMode.DoubleRow
    SQRT_HALF = float(np.sqrt(0.5))
    TERMS = ((0, 0), (0, 1), (1, 0))   # (lhs hi/lo, rhs hi/lo) cross terms

    nc = bacc.Bacc("TRN2", target_bir_lowering=False, debug=False,
                   num_devices=NCORES if with_collective else 1)

    # inputs arrive k-slab-major with hi|lo planes adjacent per row, so a
    # k-pair DMA is a clean 3-dim transfer
    xhl = nc.dram_tensor("xhl", [KT, P, 2 * NSH], E4, kind="ExternalInput")
    yhl = nc.dram_tensor("yhl", [KT, P, 2 * NSH], E4, kind="ExternalInput")
    Wxhl = nc.dram_tensor("Wxhl", [KT, P, 2 * EMB], E4, kind="ExternalInput")
    Wyhl = nc.dram_tensor("Wyhl", [KT, P, 2 * EMB], E4, kind="ExternalInput")
    Wg = nc.dram_tensor("Wg", [EMB, EMB_OUT], BF16, kind="ExternalInput")
    bx_bc = nc.dram_tensor("bx_bc", [P, EMB], F32, kind="ExternalInput")
    bxp = nc.dram_tensor("bxp", [P, ME], F32, kind="ExternalInput")
    byp = nc.dram_tensor("byp", [P, ME], F32, kind="ExternalInput")
    bgp = nc.dram_tensor("bgp", [P, EMB_OUT // P], F32, kind="ExternalInput")
    halves = nc.dram_tensor("halves", [P, P], BF16, kind="ExternalInput")
    outT = nc.dram_tensor("outT", [EMB_OUT, NSH], F32, kind="ExternalOutput")

    with tile.TileContext(nc) as tc:
        with (
            tc.tile_pool(name="perm", bufs=1) as perm,
            tc.tile_pool(name="psA", bufs=3, space="PSUM") as psA,
            tc.tile_pool(name="dramp", bufs=1, space="DRAM") as dramp,
        ):
            # ---- permanent tiles ----
            ypT_hi = perm.tile([P, ME, NSH], E4)
            ypT_lo = perm.tile([P, ME, NSH], E4)
            drow_sb = perm.tile([P, NSH], F32)       # holds Drow/2
            dcol_rot = perm.tile([P, TALL], F32)     # Dcol/2, rotated shards
            Wg_sb = perm.tile([P, ME, EMB_OUT], BF16)
            bgp_sb = perm.tile([P, EMB_OUT // P], F32)
            # own-shard projections stay resident so pass A/G can start on
            # them before the AllGather completes (shard-rotated t order)
            xp_hi = perm.tile([P, TSH, EMB], E4)
            xp_lo = perm.tile([P, TSH, EMB], E4)
            xpT_hi = perm.tile([P, ME, NSH], E4)
            xpT_lo = perm.tile([P, ME, NSH], E4)
            dcol_own = perm.tile([P, TSH], F32)      # Dcol/2, own shard

            ag_in = dramp.tile([SH_ELEMS], E4)
            ag_out = dramp.tile([NCORES * SH_ELEMS], E4, addr_space="Shared")

            ap = ag_in[:]
            xph_region = ap[0:XP_ELEMS].rearrange(
                "(p m e) -> p m e", p=P, m=TSH)
            xpl_region = ap[XP_ELEMS:2 * XP_ELEMS].rearrange(
                "(p m e) -> p m e", p=P, m=TSH)
            xpTh_region = ap[XPT_OFF:XPT_OFF + XPT_ELEMS].rearrange(
                "(p m n) -> p m n", p=P, m=ME)
            xpTl_region = ap[XPT_OFF + XPT_ELEMS:DC_OFF].rearrange(
                "(p m n) -> p m n", p=P, m=ME)
            dc_region = ap[DC_OFF:SH_ELEMS].bitcast(F32).rearrange(
                "(p m) -> p m", p=P)

            # ================= phase 1: own-shard projections ================
            with (
                tc.tile_pool(name="wpool", bufs=1) as wpool,
                tc.tile_pool(name="scr", bufs=2) as scr,
                tc.tile_pool(name="ph1ps", bufs=2, space="PSUM") as ph1ps,
            ):
                xT_sb = wpool.tile([P, KT, 2, NSH], E4)
                yT_sb = wpool.tile([P, KT, 2, NSH], E4)
                Wx_sb = wpool.tile([P, KT, 2, EMB], E4)
                Wy_sb = wpool.tile([P, KT, 2, EMB], E4)
                bx_bc_sb = wpool.tile([P, EMB], F32)
                bxp_sb = wpool.tile([P, ME], F32)
                byp_sb = wpool.tile([P, ME], F32)
                halves_sb = wpool.tile([P, P], BF16)
                # k-pair-granular input DMAs (matching DoubleRow consumption),
                # x-stream first so the xp matmuls start ~1.5us in; biases
                # and the y-stream follow; Wg/bgp (MLP-only) come last.
                xhl_ap = xhl.ap().rearrange("k p n -> p k n")
                yhl_ap = yhl.ap().rearrange("k p n -> p k n")
                Wxhl_ap = Wxhl.ap().rearrange("k p e -> p k e")
                Wyhl_ap = Wyhl.ap().rearrange("k p e -> p k e")
                # kp=0 split fine (hi planes first, xT-hi halved) so the
                # first DoubleRow matmul can start ~1.6us in
                k01 = slice(0, 2)
                nc.sync.dma_start(Wx_sb[:, k01, 0, :], Wxhl_ap[:, k01, 0:EMB])
                nc.sync.dma_start(xT_sb[:, k01, 0, 0:512],
                                  xhl_ap[:, k01, 0:512])
                nc.sync.dma_start(xT_sb[:, k01, 0, 512:NSH],
                                  xhl_ap[:, k01, 512:NSH])
                nc.sync.dma_start(Wx_sb[:, k01, 1, :],
                                  Wxhl_ap[:, k01, EMB:2 * EMB])
                nc.sync.dma_start(xT_sb[:, k01, 1, :],
                                  xhl_ap[:, k01, NSH:2 * NSH])
                for kp in range(2, KT, 2):
                    ks = slice(kp, kp + 2)
                    nc.sync.dma_start(Wx_sb[:, ks, :, :], Wxhl_ap[:, ks, :])
                    nc.sync.dma_start(xT_sb[:, ks, :, :], xhl_ap[:, ks, :])
                nc.sync.dma_start(bx_bc_sb[:], bx_bc.ap())
                nc.sync.dma_start(bxp_sb[:], bxp.ap())
                nc.sync.dma_start(byp_sb[:], byp.ap())
                nc.sync.dma_start(halves_sb[:], halves.ap())
                for kp in range(0, KT, 2):
                    ks = slice(kp, kp + 2)
                    nc.sync.dma_start(Wy_sb[:, ks, :, :], Wyhl_ap[:, ks, :])
                    nc.sync.dma_start(yT_sb[:, ks, :, :], yhl_ap[:, ks, :])
                nc.sync.dma_start(
                    Wg_sb[:], Wg.ap().rearrange("(kt p) n -> p kt n", p=P))
                nc.sync.dma_start(bgp_sb[:], bgp.ap())

                # xp shard: [128, m, 512], nx on partitions.  k-pair-major
                # across all 8 m-groups (8 concurrent PSUM banks) so PE
                # issues 24 DoubleRow matmuls per arriving xT slab pair.
                xp_grp = []
                for m in range(TSH):
                    pool_m = psA if m < 3 else ph1ps
                    tag_m = "mm" if m < 3 else "grp"
                    xp_grp.append(pool_m.tile(
                        [P, EMB], mybir.dt.float32, tag=tag_m,
                        bufs=(3 if m < 3 else 5),
                        name=f"ps_xp{m}"))
                for kp in range(0, KT, 2):
                    for m in range(TSH):
                        for ti, (hx, hw) in enumerate(TERMS):
                            nc.tensor.matmul(
                                xp_grp[m][:],
                                xT_sb[:, kp:kp + 2, hx, m * P:(m + 1) * P],
                                Wx_sb[:, kp:kp + 2, hw, :],
                                start=(kp == 0 and ti == 0),
                                stop=(kp == KT - 2 and ti == 2),
                                perf_mode=DR)
                for m in range(TSH):
                    xps = scr.tile([P, EMB], F32, tag="xps", name="xps")
                    nc.vector.scalar_tensor_tensor(
                        out=xps[:], in0=xp_grp[m][:], scalar=1.0 / (SX * SW),
                        in1=bx_bc_sb[:], op0=ALU.mult, op1=ALU.add)
                    sq = scr.tile([P, EMB], F32, tag="sq", name="sq")
                    # Square(sqrt(.5)*x) accumulated along free dim -> Dcol/2
                    nc.scalar.activation(
                        sq[:], xps[:], ACTF.Square,
                        scale=SQRT_HALF, accum_out=dcol_own[:, m:m + 1])
                    # hi casts ride the otherwise-idle Pool engine in phase 1
                    nc.gpsimd.tensor_scalar_mul(
                        xp_hi[:, m, :], xps[:], SP)
                    nc.vector.scalar_tensor_tensor(
                        out=xp_lo[:, m, :], in0=xps[:], scalar=SP,
                        in1=xp_hi[:, m, :], op0=ALU.mult, op1=ALU.subtract)
                    nc.sync.dma_start(xph_region[:, m, :],
                                      xp_hi[:, m, :])
                    nc.sync.dma_start(xpl_region[:, m, :],
                                      xp_lo[:, m, :])

                # xpT shard: [128, me, 1024], emb on partitions
                for m in range(ME):
                    for nb in range(NSH // 512):
                        ps = psA.tile([P, 512], mybir.dt.float32, tag="mm",
                                      name="ps_xpt")
                        cols = slice(nb * 512, (nb + 1) * 512)
                        for kp in range(0, KT, 2):
                            for ti, (hw, hx) in enumerate(TERMS):
                                nc.tensor.matmul(
                                    ps[:],
                                    Wx_sb[:, kp:kp + 2, hw,
                                          m * P:(m + 1) * P],
                                    xT_sb[:, kp:kp + 2, hx, cols],
                                    start=(kp == 0 and ti == 0),
                                    stop=(kp == KT - 2 and ti == 2),
                                    perf_mode=DR)
                        xts = scr.tile([P, 512], F32, tag="xts", name="xts")
                        nc.scalar.activation(
                            xts[:], ps[:], ACTF.Identity,
                            bias=bxp_sb[:, m:m + 1], scale=1.0 / (SX * SW))
                        nc.gpsimd.tensor_scalar_mul(
                            xpT_hi[:, m, cols], xts[:], SP)
                        nc.vector.scalar_tensor_tensor(
                            out=xpT_lo[:, m, cols], in0=xts[:], scalar=SP,
                            in1=xpT_hi[:, m, cols], op0=ALU.mult,
                            op1=ALU.subtract)
                # whole-tile xpT packs: 2 descriptor-generation slots instead
                # of 16, so the phase-2 stream prefetches behind them in the
                # in-order HWDGE queue start ~5us earlier
                nc.sync.dma_start(xpTh_region[:], xpT_hi[:])
                nc.sync.dma_start(xpTl_region[:], xpT_lo[:])
                nc.sync.dma_start(dc_region[:], dcol_own[:])
                if with_collective:
                    nc.gpsimd.collective_compute(
                        "AllGather", ALU.bypass,
                        replica_groups=[list(range(NCORES))],
                        ins=[ag_in[:].opt()],
                        outs=[ag_out[:].opt()],
                    )

                # ypT shard (overlaps the AllGather).  nb-outer order so the
                # sub=0 half (ypT columns 0:512 + Drow 0:512) completes
                # first and pass A can start early.  Drow/2 = colsum(ypT^2)
                # via 0.5-valued ones-matmul, broadcast to all partitions;
                # the ones-matmuls run one (nb,m) block behind the
                # yts->sqd chain so PE never waits on ACT/DVE.
                drow_ps = {}
                sq_pend = []

                def drain_sq(limit):
                    while len(sq_pend) > limit:
                        sqd_, nb_, m_ = sq_pend.pop(0)
                        nc.tensor.matmul(
                            drow_ps[nb_][:], halves_sb[:], sqd_[:],
                            start=(m_ == 0), stop=(m_ == ME - 1))
                        if m_ == ME - 1:
                            nc.vector.tensor_copy(
                                drow_sb[:, nb_ * 512:(nb_ + 1) * 512],
                                drow_ps[nb_][:])

                for nb in range(NSH // 512):
                    drow_ps[nb] = ph1ps.tile(
                        [P, 512], mybir.dt.float32, tag="grp",
                        bufs=5, name=f"drow_ps{nb}")
                    cols = slice(nb * 512, (nb + 1) * 512)
                    for m in range(ME):
                        ps = psA.tile([P, 512], mybir.dt.float32, tag="mm",
                                      name="ps_ypt")
                        for kp in range(0, KT, 2):
                            for ti, (hw, hy) in enumerate(TERMS):
                                nc.tensor.matmul(
                                    ps[:],
                                    Wy_sb[:, kp:kp + 2, hw,
                                          m * P:(m + 1) * P],
                                    yT_sb[:, kp:kp + 2, hy, cols],
                                    start=(kp == 0 and ti == 0),
                                    stop=(kp == KT - 2 and ti == 2),
                                    perf_mode=DR)
                        drain_sq(1)
                        yts = scr.tile([P, 512], F32, tag="yts", name="yts")
                        nc.scalar.activation(
                            yts[:], ps[:], ACTF.Identity,
                            bias=byp_sb[:, m:m + 1], scale=1.0 / (SX * SW))
                        nc.gpsimd.tensor_scalar_mul(
                            ypT_hi[:, m, cols], yts[:], SP)
                        nc.vector.scalar_tensor_tensor(
                            out=ypT_lo[:, m, cols], in0=yts[:], scalar=SP,
                            in1=ypT_hi[:, m, cols], op0=ALU.mult,
                            op1=ALU.subtract)
                        sqd = scr.tile([P, 512], BF16, tag="sqd", name="sqd")
                        nc.scalar.activation(
                            sqd[:], yts[:], ACTF.Square, scale=1.0)
                        sq_pend.append((sqd, nb, m))
                drain_sq(0)

            # ============== phase 2/3: gathered passes ==============
            with (
                tc.tile_pool(name="stream", bufs=1) as stream,
                tc.tile_pool(name="work", bufs=1) as work,
                tc.tile_pool(name="psG", bufs=4, space="PSUM") as psG,
            ):
                # shard-rotation: core c processes shard order
                # c, c+1, ..., c+7 (mod 8).  j=0 reads its own projections
                # straight from SBUF (no AllGather dependency); j>=1 reads
                # the gathered buffer at a partition_id-dependent offset, by
                # which time the AllGather has completed behind phase-1 work.
                import concourse.bass as bass_mod
                pid = nc.partition_id() if with_collective else 0
                bases = [None] + [
                    ((pid + j) % NCORES) * SH_ELEMS for j in range(1, NCORES)
                ]
                def load_dcol(j):
                    nc.sync.dma_start(
                        dcol_rot[:, j * TSH:(j + 1) * TSH],
                        ag_out[:][bass_mod.ds(bases[j] + DC_OFF, DCOL_SLOTS)]
                        .bitcast(F32).rearrange("(p m) -> p m", p=P))

                # j=1's dcol is needed ~10us into phase 2; the rest are
                # emitted lazily inside the t loop so their descriptor
                # generation doesn't delay the j=1 stream prefetches
                load_dcol(1)

                subs = [s for _ in range(passes_repeat) for s in range(NSUBS)]
                npairs = TALL // 2
                pending_q = []   # entries from the flat (sub, t) loop
                deferred = []    # [(ready_gt, emit_fn)] e.g. per-sub MLP

                def emit_mlp(gwf_ps_s, ycols):
                    gwfT = work.tile([P, ME, EMB], BF16, tag="gwfT", bufs=2,
                                     name="gwfT")
                    # gwf_ps holds (SA*SP)*gwf; descale on the PSUM->SBUF
                    # copies, split across ACT and DVE
                    for e in range(ME):
                        if e % 2 == 0:
                            nc.vector.tensor_scalar_mul(
                                gwfT[:, e, :], gwf_ps_s[e][:],
                                1.0 / (SA * SP))
                        else:
                            nc.scalar.activation(
                                gwfT[:, e, :], gwf_ps_s[e][:], ACTF.Identity,
                                scale=1.0 / (SA * SP))

                    def mlp():
                        for m in range(EMB_OUT // P):
                            ps2 = psA.tile([P, NYSUB], mybir.dt.float32,
                                           tag="mm", name="ps_mlp")
                            for k in range(ME):
                                nc.tensor.matmul(
                                    ps2[:], Wg_sb[:, k, m * P:(m + 1) * P],
                                    gwfT[:, k, :], start=(k == 0),
                                    stop=(k == ME - 1))
                            ot = work.tile([P, NYSUB], F32, tag="ot",
                                           bufs=2, name="ot")
                            nc.scalar.activation(
                                ot[:], ps2[:], ACTF.Relu,
                                bias=bgp_sb[:, m:m + 1], scale=1.0)
                            nc.scalar.dma_start(
                                outT.ap()[m * P:(m + 1) * P, ycols], ot[:])
                    return mlp

                def flush_one():
                    ent = pending_q.pop(0)
                    get_xh, get_xl, ah, al, pi, gwf_ps_s, ycols, gt = ent
                    for e in range(ME):
                        terms = ((get_xh, ah), (get_xh, al), (get_xl, ah))
                        for ti, (gx, aop) in enumerate(terms):
                            nc.tensor.matmul(
                                gwf_ps_s[e][:], gx(e), aop[:],
                                start=(pi == 0 and ti == 0),
                                stop=(pi == npairs - 1 and ti == 2),
                                perf_mode=DR)
                    if pi == npairs - 1:
                        deferred.append((gt + 3, emit_mlp(gwf_ps_s, ycols)))

                def flush_gwf(gt):
                    # flush pairs whose elementwise chain has had >=3
                    # t-slots (~3.8us of PE work) to complete
                    while pending_q and (gt is None
                                         or gt - pending_q[0][7] >= 3):
                        flush_one()
                    while deferred and (gt is None or deferred[0][0] <= gt):
                        deferred.pop(0)[1]()

                for si, sub in enumerate(subs):
                    ycols = slice(sub * NYSUB, (sub + 1) * NYSUB)
                    gwf_ps = [
                        psG.tile([P, NYSUB], mybir.dt.float32, tag="gwf",
                                 name=f"gwf{e}")
                        for e in range(ME)
                    ]
                    # streamed shards: one [P, ME/TSH, NSH] block per plane
                    # per shard (4 DMAs per 8 t), prefetched a shard ahead
                    shard_tiles = {}

                    def fetch_shard(jj):
                        if jj in shard_tiles or jj < 1 or jj >= NCORES:
                            return
                        xTh = stream.tile([P, ME, NSH], E4, tag="xpTbh",
                                          bufs=3, name="xpTb_h")
                        nc.sync.dma_start(
                            xTh[:],
                            ag_out[:][bass_mod.ds(
                                bases[jj] + XPT_OFF, XPT_ELEMS)]
                            .rearrange("(p m n) -> p m n", p=P, m=ME))
                        xTl = stream.tile([P, ME, NSH], E4, tag="xpTbl",
                                          bufs=3, name="xpTb_l")
                        nc.sync.dma_start(
                            xTl[:],
                            ag_out[:][bass_mod.ds(
                                bases[jj] + XPT_OFF + XPT_ELEMS, XPT_ELEMS)]
                            .rearrange("(p m n) -> p m n", p=P, m=ME))
                        xh = stream.tile([P, TSH, EMB], E4, tag="xph",
                                         bufs=3, name="xph_t")
                        nc.sync.dma_start(
                            xh[:],
                            ag_out[:][bass_mod.ds(bases[jj], XP_ELEMS)]
                            .rearrange("(p m e) -> p m e", p=P, m=TSH))
                        xl = stream.tile([P, TSH, EMB], E4, tag="xpl",
                                         bufs=3, name="xpl_t")
                        nc.sync.dma_start(
                            xl[:],
                            ag_out[:][bass_mod.ds(
                                bases[jj] + XP_ELEMS, XP_ELEMS)]
                            .rearrange("(p m e) -> p m e", p=P, m=TSH))
                        shard_tiles[jj] = (xTh, xTl, xh, xl)

                    for t in range(TALL):
                        gt = si * TALL + t
                        j, lt = t // TSH, t % TSH
                        even = (t % 2 == 0)
                        if si == 0 and t % TSH == 0 and 2 + t // TSH < NCORES:
                            load_dcol(2 + t // TSH)
                        fetch_shard(j + 1)
                        xpT_col = lt * P
                        if j == 0:
                            xT_h, xT_l = xpT_hi, xpT_lo
                            xp_h, xp_l = xp_hi, xp_lo
                            dcol_bias = dcol_own[:, lt:lt + 1]
                        else:
                            xT_h, xT_l, xp_h, xp_l = shard_tiles[j]
                            dcol_bias = dcol_rot[:, t:t + 1]

                        if even:
                            def get_xh(e, tl=xp_h, s=lt):
                                return tl[:, s:s + 2, e * P:(e + 1) * P]

                            def get_xl(e, tl=xp_l, s=lt):
                                return tl[:, s:s + 2, e * P:(e + 1) * P]
                            a_hi_pair = work.tile([P, 2, NYSUB], E4,
                                                  tag="ah", bufs=4,
                                                  name="a_hi_pair")
                            a_lo_pair = work.tile([P, 2, NYSUB], E4,
                                                  tag="al", bufs=4,
                                                  name="a_lo_pair")

                        # A_: 6 DoubleRow matmuls (3 hi/lo terms x 2 k-pairs)
                        aps = psA.tile([P, NYSUB], mybir.dt.float32, tag="mm",
                                       name="aps")
                        terms = ((xT_h, ypT_hi), (xT_h, ypT_lo),
                                 (xT_l, ypT_hi))
                        for ti, (xx, yy) in enumerate(terms):
                            for kp in (0, 2):
                                nc.tensor.matmul(
                                    aps[:],
                                    xx[:, kp:kp + 2,
                                       xpT_col:xpT_col + P],
                                    yy[:, kp:kp + 2, ycols],
                                    start=(ti == 0 and kp == 0),
                                    stop=(ti == 2 and kp == 2),
                                    perf_mode=DR)
                        flush_gwf(gt)
                        # normalization: aps holds SP^2*A_; with
                        # r = 2/(dcol+drow) from the pre-halved sums,
                        # a_f = aps*r = SA*A exactly when SA == SP^2.
                        # Engine split: ACT does d and the hi cast, DVE does
                        # recip and the A_*r multiply, Pool the lo subtract.
                        d = work.tile([P, NYSUB], F32, tag="d", bufs=4,
                                      name="d")
                        nc.scalar.activation(
                            d[:], drow_sb[:, ycols], ACTF.Identity,
                            bias=dcol_bias, scale=1.0)
                        r = work.tile([P, NYSUB], F32, tag="r", bufs=4,
                                      name="r")
                        nc.vector.reciprocal_approx_fast(out=r[:], in_=d[:])
                        a_f = work.tile([P, NYSUB], F32, tag="af", bufs=4,
                                        name="a_f")
                        nc.vector.tensor_tensor(
                            a_f[:], aps[:], r[:], ALU.mult)
                        sl = t % 2
                        nc.scalar.activation(
                            a_hi_pair[:, sl, :], a_f[:], ACTF.Identity,
                            scale=1.0)
                        nc.gpsimd.tensor_tensor(
                            a_lo_pair[:, sl, :], a_f[:], a_hi_pair[:, sl, :],
                            ALU.subtract)
                        if not even:
                            pending_q.append(
                                (get_xh, get_xl, a_hi_pair, a_lo_pair,
                                 t // 2, gwf_ps, ycols, gt))
                flush_gwf(None)
    nc.compile()
    return nc


def _get_runner():
    """Compile once and return the jitted 8-core runner + metadata."""
    if "runner" in _CACHE:
        return _CACHE["runner"]

    import jax
    import concourse.mybir as mybir
    from concourse import bass2jax
    from concourse.bass2jax import _bass_exec_p, install_neuronx_cc_hook
    from jax.experimental.shard_map import shard_map
    from jax.sharding import Mesh, PartitionSpec

    nc = _build_nc()
    install_neuronx_cc_hook()

    partition_name = (nc.partition_id_tensor.name
                      if nc.partition_id_tensor else None)
    in_names, out_names, out_avals = [], [], []
    for alloc in nc.m.functions[0].allocations:
        if not isinstance(alloc, mybir.MemoryLocationSet):
            continue
        name = alloc.memorylocations[0].name
        if alloc.kind == "ExternalInput":
            if name != partition_name:
                in_names.append(name)
        elif alloc.kind == "ExternalOutput":
            out_names.append(name)
            out_avals.append(jax.core.ShapedArray(
                tuple(alloc.tensor_shape), mybir.dt.np(alloc.dtype)))
    n_params = len(in_names)
    n_outs = len(out_names)
    all_names = in_names + out_names
    if partition_name is not None:
        all_names = all_names + [partition_name]

    def _body(*args):
        operands = list(args)
        if partition_name is not None:
            operands.append(bass2jax.partition_id_tensor())
        outs = _bass_exec_p.bind(
            *operands,
            out_avals=tuple(out_avals),
            in_names=tuple(all_names),
            out_names=tuple(out_names),
            lowering_input_output_aliases=(),
            sim_require_finite=True,
            sim_require_nnan=True,
            nc=nc,
        )
        return tuple(outs)

    devices = jax.devices()[:NCORES]
    mesh = Mesh(np.asarray(devices), ("core",))
    specs = (PartitionSpec("core"),) * (n_params + n_outs)
    donate = tuple(range(n_params, n_params + n_outs))
    sharded = jax.jit(
        shard_map(_body, mesh=mesh, in_specs=specs,
                  out_specs=(PartitionSpec("core"),) * n_outs, check_rep=False),
        donate_argnums=donate, keep_unused=True,
    )
    runner = {
        "f": sharded, "in_names": in_names, "out_names": out_names,
        "out_shapes": [tuple(a.shape) for a in out_avals],
        "out_dtypes": [a.dtype for a in out_avals],
    }
    _CACHE["runner"] = runner
    return runner


def _pack_khl(aT, scale, e4):
    """[F, N] -> [KT, P, 2N] e4m3: k-slab-major with hi|lo adjacent rows."""
    s = np.asarray(aT, np.float32) * scale
    hi = s.astype(e4)
    lo = (s - hi.astype(np.float32)).astype(e4)
    n = aT.shape[1]
    kt = aT.shape[0] // P
    return np.concatenate(
        [hi.reshape(kt, P, n), lo.reshape(kt, P, n)], axis=2)


def _host_prep(x, y, Wx, bx, Wy, by, Wg, bg):
    """Build the concatenated (8*dim0, ...) global input arrays."""
    import ml_dtypes

    e4 = ml_dtypes.float8_e4m3
    x = np.ascontiguousarray(x, dtype=np.float32)
    y = np.ascontiguousarray(y, dtype=np.float32)
    xT = x.T  # [FX, NX]
    yT = y.T
    bx_bc = np.tile(np.asarray(bx, np.float32)[None, :], (P, 1))
    bxp = np.asarray(bx, np.float32).reshape(ME, P).T.copy()
    byp = np.asarray(by, np.float32).reshape(ME, P).T.copy()
    bgp = np.asarray(bg, np.float32).reshape(EMB_OUT // P, P).T.copy()
    halves = np.full((P, P), 0.5, ml_dtypes.bfloat16)
    Wx_hl = _pack_khl(Wx, SW, e4)
    Wy_hl = _pack_khl(Wy, SW, e4)

    per_core = {
        "xhl": [_pack_khl(xT[:, c * NSH:(c + 1) * NSH], SX, e4)
                for c in range(NCORES)],
        "yhl": [_pack_khl(yT[:, c * NSH:(c + 1) * NSH], SX, e4)
                for c in range(NCORES)],
        "Wxhl": [Wx_hl] * NCORES,
        "Wyhl": [Wy_hl] * NCORES,
        "Wg": [np.asarray(Wg, np.float32).astype(ml_dtypes.bfloat16)] * NCORES,
        "bx_bc": [bx_bc] * NCORES,
        "bxp": [bxp] * NCORES,
        "byp": [byp] * NCORES,
        "bgp": [bgp] * NCORES,
        "halves": [halves] * NCORES,
    }
    runner = _get_runner()
    concat = [np.concatenate(per_core[name], axis=0)
              for name in runner["in_names"]]
    zeros = [np.zeros((NCORES * s[0],) + s[1:], d)
             for s, d in zip(runner["out_shapes"], runner["out_dtypes"])]
    return concat, zeros


def kernel(x, y, Wx, bx, Wy, by, Wg, bg):
    concat, zeros = _host_prep(x, y, Wx, bx, Wy, by, Wg, bg)
    runner = _get_runner()
    out_arrs = runner["f"](*concat, *zeros)
    idx = runner["out_names"].index("outT")
    outT_all = np.asarray(out_arrs[idx]).reshape(NCORES, EMB_OUT, NSH)
    out = np.empty((NY, EMB_OUT), np.float32)
    for c in range(NCORES):
        out[c * NSH:(c + 1) * NSH, :] = outT_all[c].T
    return out
